# revision 35
# baseline (speedup 1.0000x reference)
"""Trainium2 Bass kernel for nn_AttentionSimilarity (v18 final).

Reference computation (per batch element, B=8 data-parallel over 8 cores):
    q_in = pairwise-mean(x)            # [M, D], M = N/2
    q    = q_in @ Wq.T + bq            # [M, D]
    k    = x @ Wk.T + bk               # [N, D]
    v    = x @ Wv.T + bv               # [N, D]
    attn = softmax(q @ k.T / sqrt(D))  # [M, N]
    o    = attn @ v                    # [M, D]
    return (q, o, o)

Algebra (all exact):
  * S = q @ k.T = (q @ Wk) @ x.T + (q.bk) 1^T; the q.bk term is constant
    along n so it cancels in softmax -> k is never materialized. Instead
    G = q @ Wk ([M, D]) is computed from qT (8x fewer MACs than k).
  * v-projection reassociated: o = (P @ x) @ Wv.T + bv; row-normalization
    commutes with the Wv projection: o = (PX @ Wv.T)/r + bv, so the
    epilogue PE work never waits on the row sums.

Schedule notes:
  * Engine load-balance: PE does only matmuls/transposes; ACT evicts
    qts/gts psums + exp; GpSimd does f32->bf16 input conversions; DVE does
    batched transpose evictions, pooling, PX folds and o normalization.
  * Transpose psum evictions are batched 8-at-a-time into one PSUM bank
    ([P, 8*128]) -> single DVE copy, to dodge per-instruction overhead.
  * Two DMA queues: x-in + BC loads on SP; weight-in and all outputs on
    the Activation HWDGE queue.
  * Weight setup is interleaved with the first strips so PE never idles
    waiting on weight DMAs.

Softmax max-subtraction is skipped: logits are ~N(0, 0.7^2) (plus a small
per-row shift from dropping q.bk) so exp is safe in fp32.
"""

import sys

if "/opt/trn_rl_repo" not in sys.path:
    sys.path.insert(0, "/opt/trn_rl_repo")

from contextlib import ExitStack

import numpy as np

import concourse.bass as bass
import concourse.mybir as mybir
import concourse.tile as tile
from concourse import bacc
from concourse.masks import make_identity

F32 = mybir.dt.float32
BF = mybir.dt.bfloat16
AF = mybir.ActivationFunctionType
P = 128

# Full-problem constants
FULL_B, FULL_N, FULL_D = 8, 4096, 1024


def build_program(
    N=FULL_N,
    D=FULL_D,
    repeats=1,
    tune=None,
    **tune_kw,
):
    """Build the per-core SPMD Bass program. Every core runs the same
    program on its own batch element; no collectives."""
    M = N // 2
    DC = D // P        # feature chunks of 128 (8)
    NC = N // P        # key chunks of 128 (32)
    SCALE = float(D) ** -0.5

    T = dict(
        nstrip=256,     # x rows per phase-A strip
        msup=512,       # m columns per BC super-block
        grp=8,          # n-chunks per PX psum accumulation group
        xn_bufs=2, xq_bufs=2, qts_bufs=1, gts_bufs=1, qev_bufs=2,
        gtc_bufs=2, pt_bufs=1, pxt_bufs=2, oout_bufs=3,
        work_ps=3, px_ps=2, kq_ps=2, t_ps=2, qev_ps=2,
    )
    if tune:
        T.update(tune)
    if tune_kw:
        T.update(tune_kw)

    nstrip = T["nstrip"]
    SJ = nstrip // P   # 128-row blocks per strip (2)
    SN = N // nstrip   # strips (16)
    MSUP = T["msup"]
    GM = M // MSUP     # super-blocks (4)
    MB = MSUP // P     # m sub-blocks per super (4)
    QB = 2             # strips per phase-A q-batch
    G = T["grp"]
    NG = NC // G       # psum groups per super (4)
    mw = nstrip // 2   # pooled columns per strip (128)
    BW = QB * mw       # m columns per q-batch (256)

    nc = bacc.Bacc("TRN2", target_bir_lowering=False, debug=False)

    x_d = nc.dram_tensor("x", [N, D], F32, kind="ExternalInput").ap()
    wq_d = nc.dram_tensor("Wq", [D, D], F32, kind="ExternalInput").ap()
    bq_d = nc.dram_tensor("bq", [D], F32, kind="ExternalInput").ap()
    wk_d = nc.dram_tensor("Wk", [D, D], F32, kind="ExternalInput").ap()
    bk_d = nc.dram_tensor("bk", [D], F32, kind="ExternalInput").ap()
    wv_d = nc.dram_tensor("Wv", [D, D], F32, kind="ExternalInput").ap()
    bv_d = nc.dram_tensor("bv", [D], F32, kind="ExternalInput").ap()
    q_d = nc.dram_tensor("q", [M, D], F32, kind="ExternalOutput").ap()
    o_d = nc.dram_tensor("o", [M, D], F32, kind="ExternalOutput").ap()
    del bk_d  # unused: q.bk is constant along n, cancels in softmax

    def mm(ps, lhsT, rhs, start, stop):
        nc.tensor.matmul(ps, lhsT, rhs, start=start, stop=stop)

    with tile.TileContext(nc) as tc, ExitStack() as ctx:
        const = ctx.enter_context(tc.tile_pool(name="const", bufs=1))
        dram = ctx.enter_context(tc.tile_pool(name="dram", bufs=1, space="DRAM"))

        SUPB = 512 // (2 * (256 // 2))  # batches per super (msup/BW), fixed below
        gT_drams = [dram.tile([P, DC, 512], BF, name=f"gT{g}")
                    for g in range(M // 512)]
        wvT_dram = dram.tile([P, DC, D], BF)

        identity = const.tile([P, P], F32)
        make_identity(nc, identity)
        identity_bf = const.tile([P, P], BF)
        nc.vector.tensor_copy(identity_bf, identity)
        ones2_f32 = const.tile([P, 2], F32)
        nc.vector.memset(ones2_f32, 1.0)
        ones2_bf = const.tile([P, 2], BF)
        nc.vector.tensor_copy(ones2_bf, ones2_f32)
        ones_row = const.tile([1, P], F32)
        nc.vector.memset(ones_row, 1.0)
        bv_bc = const.tile([P, D], BF)

        # ---- residents ----
        res_pool = ctx.enter_context(tc.tile_pool(name="res", bufs=1))
        xT_sb = res_pool.tile([P, DC, N], BF)     # x^T: [d-part, dc, n]
        x_sb = res_pool.tile([P, NC, D], BF)      # x:   [n-part, nchunk, d]

        # ---- bq per-partition layout [P, DC]: bq_sb[p, c] = bq[c*128+p] ----
        with ExitStack() as bias_ctx:
            bpsum = bias_ctx.enter_context(
                tc.tile_pool(name="bpsum", bufs=1, space="PSUM"))
            brow_pool = bias_ctx.enter_context(tc.tile_pool(name="brow", bufs=1))
            brow = brow_pool.tile([DC, P], F32, tag="brow")
            nc.scalar.dma_start(brow, bq_d.rearrange("(c p) -> c p", p=P))
            ps = bpsum.tile([P, DC], F32, tag="bps")
            nc.tensor.transpose(ps, brow, identity[:DC, :DC])
            bq_sb = const.tile([P, DC], F32, tag="bias_bq")
            nc.vector.tensor_copy(bq_sb, ps)
            # bv broadcast [P, D] via K=1 ones-matmul, built once
            brow2 = brow_pool.tile([1, D], F32, tag="brow_flat")
            nc.scalar.dma_start(brow2, bv_d[None, :])
            for gh in range(2):
                bps2 = bpsum.tile([P, D // 2], F32, tag="bbc")
                nc.tensor.matmul(
                    bps2, ones_row, brow2[:, gh * (D // 2):(gh + 1) * (D // 2)],
                    start=True, stop=True)
                nc.vector.tensor_copy(
                    bv_bc[:, gh * (D // 2):(gh + 1) * (D // 2)], bps2)

        for _rep in range(repeats):
            tc.no_sync_barrier()
            # =================== Phase A ===================
            with ExitStack() as actx:
                w_pool = actx.enter_context(tc.tile_pool(name="wts", bufs=1))
                tpsum = actx.enter_context(
                    tc.tile_pool(name="tpsum", bufs=T["t_ps"], space="PSUM"))
                wnat_pool = actx.enter_context(tc.tile_pool(name="wnat", bufs=2))
                wbf_pool = actx.enter_context(tc.tile_pool(name="wbf", bufs=1))
                wvs_pool = actx.enter_context(tc.tile_pool(name="wvs", bufs=1))
                xn_pool = actx.enter_context(
                    tc.tile_pool(name="xn", bufs=T["xn_bufs"]))
                xq_pool = actx.enter_context(
                    tc.tile_pool(name="xq", bufs=T["xq_bufs"]))
                qts_pool = actx.enter_context(
                    tc.tile_pool(name="qts", bufs=T["qts_bufs"]))
                gts_pool = actx.enter_context(
                    tc.tile_pool(name="gts", bufs=T["gts_bufs"]))
                qev_pool = actx.enter_context(
                    tc.tile_pool(name="qev", bufs=T["qev_bufs"]))
                qpsum = actx.enter_context(
                    tc.tile_pool(name="qpsum", bufs=T["kq_ps"], space="PSUM"))
                qev_psum = actx.enter_context(
                    tc.tile_pool(name="qev_ps", bufs=T["qev_ps"], space="PSUM"))
                gpsum = actx.enter_context(
                    tc.tile_pool(name="gpsum", bufs=T["kq_ps"], space="PSUM"))

                wqT = w_pool.tile([P, DC, D], BF, tag="wqT")
                wkN = w_pool.tile([P, DC, D], BF, tag="wkN")

                def wq_setup(e0, e1):
                    # wqT[p, dc, e] = 0.5*Wq[e, dc*128+p] (pair-mean folded)
                    for ec in range(e0, e1):
                        wn = wnat_pool.tile([P, D], F32, tag="wnat")
                        nc.scalar.dma_start(wn, wq_d[ec * P:(ec + 1) * P, :])
                        wb = wbf_pool.tile([P, D], BF, tag="wbf")
                        nc.vector.tensor_scalar_mul(wb, wn, 0.5)
                        tp = tpsum.tile([P, DC, P], BF, tag="tps")
                        for dc in range(DC):
                            nc.tensor.transpose(
                                tp[:, dc, :], wb[:, dc * P:(dc + 1) * P],
                                identity_bf)
                        nc.vector.tensor_copy(
                            wqT[:, :, ec * P:(ec + 1) * P], tp)

                def wk_setup():
                    # wkN[p, ec, d] = Wk[ec*128+p, d] (natural, bf16)
                    for ec in range(DC):
                        wn = wnat_pool.tile([P, D], F32, tag="wnat")
                        nc.scalar.dma_start(wn, wk_d[ec * P:(ec + 1) * P, :])
                        nc.scalar.activation(wkN[:, ec, :], wn, AF.Identity)

                def wv_setup(e0, e1):
                    # wvT -> DRAM scratch (reloaded in phase BC)
                    for ec in range(e0, e1):
                        wn = wnat_pool.tile([P, D], F32, tag="wnat")
                        nc.scalar.dma_start(wn, wv_d[ec * P:(ec + 1) * P, :])
                        wb = wbf_pool.tile([P, D], BF, tag="wbf")
                        nc.scalar.activation(wb, wn, AF.Identity)
                        tp = tpsum.tile([P, DC, P], BF, tag="tps")
                        for dc in range(DC):
                            nc.tensor.transpose(
                                tp[:, dc, :], wb[:, dc * P:(dc + 1) * P],
                                identity_bf)
                        wvs = wvs_pool.tile([P, DC, P], BF, tag="wvs")
                        nc.vector.tensor_copy(wvs, tp)
                        nc.scalar.dma_start(
                            wvT_dram[:, :, ec * P:(ec + 1) * P], wvs)

                xq_tiles = {}

                def strip(s):
                    for j in range(SJ):
                        i = s * SJ + j  # n-chunk index
                        xn = xn_pool.tile([P, D], F32, tag="xn")
                        xeng = nc.sync if i % 2 == 0 else nc.gpsimd
                        xeng.dma_start(xn, x_d[i * P:(i + 1) * P, :])
                        if i % 3 == 0:
                            nc.gpsimd.tensor_copy(x_sb[:, i, :], xn)
                        elif i % 3 == 1:
                            nc.scalar.activation(x_sb[:, i, :], xn, AF.Identity)
                        else:
                            nc.vector.tensor_copy(x_sb[:, i, :], xn)
                        tp = tpsum.tile([P, DC, P], BF, tag="tps")
                        for dc in range(DC):
                            nc.tensor.transpose(
                                tp[:, dc, :],
                                x_sb[:, i, dc * P:(dc + 1) * P], identity_bf)
                        nc.vector.tensor_copy(
                            xT_sb[:, :, i * P:(i + 1) * P], tp)
                    # adjacent-pair pooling (0.5 folded into Wq already)
                    if s % QB == 0:
                        xq_tiles[s // QB] = xq_pool.tile(
                            [P, DC, BW], BF, tag="xq", name="xq")
                    v2 = xT_sb[:, :, s * nstrip:(s + 1) * nstrip].rearrange(
                        "p c (m two) -> p c m two", two=2)
                    nc.vector.tensor_add(
                        xq_tiles[s // QB][:, :, (s % QB) * mw:(s % QB + 1) * mw],
                        v2[:, :, :, 0], v2[:, :, :, 1])

                def batch(b):
                    moff = b * BW
                    xq = xq_tiles.pop(b)
                    qts = qts_pool.tile([P, DC, BW], BF, tag="qts")
                    for ec in range(DC):
                        qps = qpsum.tile([P, BW], F32, tag="qps")
                        for dc in range(DC):
                            mm(qps, wqT[:, dc, ec * P:(ec + 1) * P],
                               xq[:, dc, :],
                               start=(dc == 0), stop=(dc == DC - 1))
                        nc.scalar.activation(
                            qts[:, ec, :], qps, AF.Identity,
                            bias=bq_sb[:, ec:ec + 1])
                    # GT[p, dc, m] = G[m, dc*128+p], G = q @ Wk
                    gts = gts_pool.tile([P, DC, BW], BF, tag="gts")
                    for dh in range(DC // 2):
                        gps = gpsum.tile([P, 2, BW], F32, tag="gps")
                        for g2 in range(2):
                            dc = 2 * dh + g2
                            for ec in range(DC):
                                mm(gps[:, g2, :],
                                   wkN[:, ec, dc * P:(dc + 1) * P],
                                   qts[:, ec, :],
                                   start=(ec == 0), stop=(ec == DC - 1))
                        nc.scalar.activation(
                            gts[:, 2 * dh:2 * dh + 2, :], gps, AF.Identity)
                    sup, soff = divmod(moff, MSUP)
                    nc.gpsimd.dma_start(
                        gT_drams[sup][:, :, soff:soff + BW], gts)
                    # q natural: PE-transpose qts chunks -> [m, e] -> DMA
                    for jj in range(QB):
                        tp = qev_psum.tile([P, DC, P], BF, tag="qtp")
                        for ec in range(DC):
                            nc.tensor.transpose(
                                tp[:, ec, :], qts[:, ec, jj * P:(jj + 1) * P],
                                identity_bf)
                        qev = qev_pool.tile([P, DC, P], F32, tag="qev")
                        nc.vector.tensor_copy(qev, tp)
                        nc.gpsimd.dma_start(
                            q_d[moff + jj * P:moff + (jj + 1) * P, :],
                            qev.rearrange("p c w -> p (c w)"))

                # skewed schedule: batch(b) is emitted one strip-pair after
                # its inputs so PE always has queued transpose work while the
                # DVE eviction/pool chain for the next xq completes
                strip(0)
                wq_setup(0, DC // 2)
                strip(1)
                wq_setup(DC // 2, DC)
                wk_setup()
                strip(2)
                strip(3)
                batch(0)
                for s in range(4, SN):
                    strip(s)
                    if 4 <= s < 4 + DC // 2:
                        wv_setup(2 * (s - 4), 2 * (s - 4) + 2)
                    if s % QB == QB - 1:
                        batch(s // QB - 1)
                batch(SN // QB - 1)

            # =================== Phase BC (fused attention) ===================
            with ExitStack() as bctx:
                wv_pool = bctx.enter_context(tc.tile_pool(name="wvsb", bufs=1))
                gtc_pool = bctx.enter_context(
                    tc.tile_pool(name="gtc", bufs=T["gtc_bufs"]))
                pt_pool = bctx.enter_context(
                    tc.tile_pool(name="pt", bufs=T["pt_bufs"]))
                pxacc_pool = bctx.enter_context(tc.tile_pool(name="pxacc", bufs=1))
                pxt_pool = bctx.enter_context(
                    tc.tile_pool(name="pxt", bufs=T["pxt_bufs"]))
                oout_pool = bctx.enter_context(
                    tc.tile_pool(name="oout", bufs=T["oout_bufs"]))
                rinv_pool = bctx.enter_context(tc.tile_pool(name="rinv", bufs=2))
                work_ps = bctx.enter_context(
                    tc.tile_pool(name="work_ps", bufs=T["work_ps"], space="PSUM"))
                px_ps = bctx.enter_context(
                    tc.tile_pool(name="px_ps", bufs=T["px_ps"], space="PSUM"))
                r_ps_pool = bctx.enter_context(
                    tc.tile_pool(name="r_ps", bufs=1, space="PSUM"))

                # super-0's gtc first on the SP queue, then wvT; per-super
                # gT tiles keep gtc0's dependency narrow so its load
                # overlaps the tail of phase A
                gtc0 = gtc_pool.tile([P, DC, MSUP], BF, tag="gtc")
                nc.scalar.dma_start(gtc0, gT_drams[0][:, :, 0:MSUP])
                wvT = wv_pool.tile([P, DC, D], BF)
                nc.scalar.dma_start(wvT, wvT_dram)

                for g in range(GM):
                    m0 = g * MSUP
                    if g == 0:
                        gtc = gtc0
                    else:
                        gtc = gtc_pool.tile([P, DC, MSUP], BF, tag="gtc")
                        nc.scalar.dma_start(gtc, gT_drams[g][:, :, 0:MSUP])

                    # attention: S^T -> exp -> PX psum per chunk-group,
                    # folded into a bf16 SBUF accumulator (frees PSUM so the
                    # S matmuls can run at free-dim 512)
                    pxacc = pxacc_pool.tile([P, MB, D], BF, tag="pxacc")
                    r_ps = r_ps_pool.tile([P, 2 * MB], F32)
                    for g4 in range(NG):
                        ptg = pt_pool.tile([P, G, MSUP], BF, tag="pt")
                        for c in range(G):
                            i = g4 * G + c
                            sps = work_ps.tile([P, MSUP], F32, tag="sps")
                            for dc in range(DC):
                                mm(sps, xT_sb[:, dc, i * P:(i + 1) * P],
                                   gtc[:, dc, :],
                                   start=(dc == 0), stop=(dc == DC - 1))
                            nc.scalar.activation(
                                ptg[:, c, :], sps, AF.Exp, scale=SCALE)
                        for j in range(MB):
                            pxp = px_ps.tile([P, D], F32, tag="px", name="pxp")
                            for c in range(G):
                                i = g4 * G + c
                                ptj = ptg[:, c, j * P:(j + 1) * P]
                                for h in range(2):
                                    mm(pxp[:, h * (D // 2):(h + 1) * (D // 2)],
                                       ptj,
                                       x_sb[:, i, h * (D // 2):(h + 1) * (D // 2)],
                                       start=(c == 0), stop=(c == G - 1))
                                mm(r_ps[:, 2 * j:2 * j + 2], ptj, ones2_bf,
                                   start=(g4 == 0 and c == 0 and j == 0),
                                   stop=(g4 == NG - 1 and c == G - 1
                                         and j == MB - 1))
                            if g4 == 0:
                                nc.vector.tensor_copy(pxacc[:, j, :], pxp)
                            else:
                                nc.vector.tensor_add(
                                    pxacc[:, j, :], pxacc[:, j, :], pxp)

                    # epilogue: transpose pxacc (unnormalized), project with
                    # WvT, then scale by 1/r and add bv on DVE
                    for j in range(MB):
                        w = work_ps.tile([P, MSUP], F32, tag="sps", name="otp")
                        tp = w.bitcast(BF).rearrange(
                            "p (c w) -> p c w", w=P)
                        for dc in range(DC):
                            nc.tensor.transpose(
                                tp[:, dc, :],
                                pxacc[:, j, dc * P:(dc + 1) * P], identity_bf)
                        pxt = pxt_pool.tile([P, DC, P], BF, tag="pxt")
                        nc.vector.tensor_copy(pxt, tp)
                        ops = px_ps.tile([P, D], F32, tag="px")
                        # dc outer so each pxt[dc] stationary loads once
                        for dc in range(DC):
                            for h in range(2):
                                mm(ops[:, h * (D // 2):(h + 1) * (D // 2)],
                                   pxt[:, dc, :],
                                   wvT[:, dc, h * (D // 2):(h + 1) * (D // 2)],
                                   start=(dc == 0), stop=(dc == DC - 1))
                        rinv = rinv_pool.tile([P, 1], F32, tag="rinv")
                        nc.vector.reciprocal(rinv, r_ps[:, 2 * j:2 * j + 1])
                        otmp = oout_pool.tile([P, D], F32, tag="otmp")
                        oout = oout_pool.tile([P, D], F32, tag="oout")
                        late = g == GM - 1 and j >= MB - 2
                        # half-split so ACT/(DVE|GP)/DMA pipeline per output
                        for hh in range(2):
                            sl = slice(hh * (D // 2), (hh + 1) * (D // 2))
                            nc.scalar.activation(otmp[:, sl], ops[:, sl],
                                                 AF.Identity,
                                                 scale=rinv[:, 0:1])
                            if late:
                                nc.vector.tensor_add(oout[:, sl], otmp[:, sl],
                                                     bv_bc[:, sl])
                            else:
                                nc.gpsimd.tensor_add(oout[:, sl], otmp[:, sl],
                                                     bv_bc[:, sl])
                            nc.sync.dma_start(
                                o_d[m0 + j * P:m0 + (j + 1) * P, sl],
                                oout[:, sl])

    nc.compile()
    return nc


_program_cache = {}


def _get_program(key=("full",), **kwargs):
    if key not in _program_cache:
        _program_cache[key] = build_program(**kwargs)
    return _program_cache[key]


def run(inputs, trace=False, **build_kwargs):
    """inputs: dict with full-shape arrays. Returns (results, BassKernelResults)."""
    from concourse.bass_utils import run_bass_kernel_spmd

    x = np.ascontiguousarray(np.asarray(inputs["x"], dtype=np.float32))
    B = x.shape[0]
    weights = {
        k: np.ascontiguousarray(np.asarray(inputs[k], dtype=np.float32))
        for k in ("Wq", "bq", "Wk", "bk", "Wv", "bv")
    }
    key = ("full",) if not build_kwargs else tuple(sorted(build_kwargs.items()))
    nc = _get_program(key=key, **build_kwargs)
    in_maps = [dict(x=x[i], **weights) for i in range(B)]
    res = run_bass_kernel_spmd(nc, in_maps, list(range(B)), trace=trace)
    q = np.stack([res.results[i]["q"] for i in range(B)])
    o = np.stack([res.results[i]["o"] for i in range(B)])
    return (q, o), res


def kernel(x, Wq, bq, Wk, bk, Wv, bv):
    (q, o), _ = run(dict(x=x, Wq=Wq, bq=bq, Wk=Wk, bk=bk, Wv=Wv, bv=bv))
    return (q, o, o)
